# revision 1
# baseline (speedup 1.0000x reference)
"""E3nn interaction (gnn message passing) Bass kernel for 8 Trainium2 cores.

Strategy: edges are sorted by receiver and partitioned so core i owns the
segment-sum for nodes [2560*i, 2560*(i+1)).  Each core redundantly computes
the up-projected node table (fp16, partition-major for contiguous DMA) into
its own DRAM, with the radial MLP for all edges interleaved under phase A's
DMA.  Phase B streams edges in superchunks of 1024 (8 chunks of 128): one
batched dma_gather of sender rows per superchunk (alternating SWDGE queues),
host-prescaled one-hot rows arriving by plain DMA, then per chunk just 3 DVE
multiplies build the rhs products and 8 PE matmuls (attr-scaled one-hot lhsT)
realize the weighted tensor product + receiver scatter in PSUM.  Per
128-node tile the accumulator is transposed on PE, the final linear applied,
and the result DMAed straight from PSUM; the 1o interleave is undone on host.
"""
import math
import os
import numpy as np

N_NODES = 20000
N_EDGES = 200000
MUL = 128
P = 128
NCORES = 8
TILES_PER_CORE = 20
NODES_PER_CORE = TILES_PER_CORE * P          # 2560
NODE_PAD = NCORES * NODES_PER_CORE           # 20480
N_NODE_TILES = NODE_PAD // P                 # 160
N_RADIAL = 8
HIDDEN = 64
SC = 8                                       # chunks per superchunk
ESC = SC * P                                 # 1024 edges per superchunk

_CACHE = {}


def _build(c_prof):
    import concourse.bacc as bacc
    import concourse.bass as bass
    import concourse.tile as tile
    from concourse import mybir

    f16, f32, i16 = mybir.dt.float16, mybir.dt.float32, mybir.dt.int16
    MUL_ = mybir.AluOpType.mult
    SILU = mybir.ActivationFunctionType.Silu

    nch = sum(c_prof)
    assert nch % SC == 0
    nsc = nch // SC

    sched = []
    for t, n in enumerate(c_prof):
        for ci in range(n):
            sched.append((t, ci, n))

    nc = bacc.Bacc(num_swdge_queues=2)
    # nfTP[p, g, b, n] = block-transposed node feats, per-partition contiguous
    nfT = nc.declare_dram_parameter("nfT", [P, N_NODE_TILES // 4, 4, 512], f16,
                                    isOutput=False)
    wup = nc.declare_dram_parameter("wup", [P, 512], f16, isOutput=False)
    # MLP weights duplicated at partition bases 0 and 64
    w1d = nc.declare_dram_parameter("w1d", [P, HIDDEN], f16, isOutput=False)
    w2d = nc.declare_dram_parameter("w2d", [P, HIDDEN], f16, isOutput=False)
    w3d = nc.declare_dram_parameter("w3d", [P, HIDDEN], f16, isOutput=False)
    w4d = nc.declare_dram_parameter("w4d", [P, 512], f16, isOutput=False)
    wlind = nc.declare_dram_parameter("wlind", [P, 512], f16, isOutput=False)
    identd = nc.declare_dram_parameter("identd", [P, P], f16, isOutput=False)
    # attr-prescaled one-hot rows: ohd[s, p, ((j*4)+b)*128 + c] = y_b at rloc
    ohd = nc.declare_dram_parameter("ohd", [nsc, P, SC * 4 * P], f16,
                                    isOutput=False)
    eftd = nc.declare_dram_parameter("eftd", [nsc, 2 * N_RADIAL, ESC // 2], f16,
                                     isOutput=False)
    idxd = nc.declare_dram_parameter("idxd", [nsc, P, ESC // 16], i16,
                                     isOutput=False)
    outd = nc.declare_dram_parameter("outd", [NODES_PER_CORE, 512], f32,
                                     isOutput=True)

    with tile.TileContext(nc) as tc:
        with tc.tile_pool(name="const", bufs=1) as cp, \
             tc.tile_pool(name="dram", bufs=1, space="DRAM") as dp, \
             tc.tile_pool(name="upsb", bufs=4) as up_sb, \
             tc.tile_pool(name="scdat", bufs=3) as scp, \
             tc.tile_pool(name="mlp", bufs=nsc) as mp, \
             tc.tile_pool(name="mlp2", bufs=2) as mp2, \
             tc.tile_pool(name="chnk", bufs=4) as chp, \
             tc.tile_pool(name="flush", bufs=2) as fp, \
             tc.tile_pool(name="psA", bufs=1, space="PSUM") as psA, \
             tc.tile_pool(name="psW", bufs=2, space="PSUM") as psW, \
             tc.tile_pool(name="psH", bufs=2, space="PSUM") as psH, \
             tc.tile_pool(name="psF", bufs=1, space="PSUM") as psF:

            # tableP[p, nt, :] = up-projected features of node nt*128+p;
            # gather indices are host-remapped to (n%128)*160 + n//128
            table = dp.tile([P, N_NODE_TILES, 512], f16)

            wup_t = cp.tile([P, 512], f16)
            nc.sync.dma_start(out=wup_t[:], in_=wup[:])
            w1_t = cp.tile([P, HIDDEN], f16)
            nc.sync.dma_start(out=w1_t[:], in_=w1d[:])
            w2_t = cp.tile([P, HIDDEN], f16)
            nc.sync.dma_start(out=w2_t[:], in_=w2d[:])
            w3_t = cp.tile([P, HIDDEN], f16)
            nc.sync.dma_start(out=w3_t[:], in_=w3d[:])
            w4_t = cp.tile([P, 512], f16)
            nc.sync.dma_start(out=w4_t[:], in_=w4d[:])
            wlin_t = cp.tile([P, 512], f16)
            nc.sync.dma_start(out=wlin_t[:], in_=wlind[:])
            ident_t = cp.tile([P, P], f16)
            nc.sync.dma_start(out=ident_t[:], in_=identd[:])
            zt = cp.tile([P, P], f16)
            nc.vector.memset(zt[:], 0.0)

            # ---- Phase A: up-projection table with radial MLP interleaved ----
            def emit_mlp(s):
                eft = mp2.tile([P, ESC // 2], f16, tag="eft")
                nc.sync.dma_start(out=eft[0:N_RADIAL, :],
                                  in_=eftd[s, 0:N_RADIAL, :])
                nc.sync.dma_start(out=eft[64:64 + N_RADIAL, :],
                                  in_=eftd[s, N_RADIAL:2 * N_RADIAL, :])
                hp1 = psH.tile([P, 512], f32, tag="hp")
                nc.tensor.matmul(out=hp1[0:64, :], lhsT=w1_t[0:N_RADIAL, :],
                                 rhs=eft[0:N_RADIAL, :], start=True, stop=True)
                nc.tensor.matmul(out=hp1[64:128, :],
                                 lhsT=w1_t[64:64 + N_RADIAL, :],
                                 rhs=eft[64:64 + N_RADIAL, :],
                                 start=True, stop=True)
                h1 = mp2.tile([P, 512], f16, tag="h1")
                nc.scalar.activation(out=h1[:], in_=hp1[:], func=SILU)
                hp2 = psH.tile([P, 512], f32, tag="hp")
                nc.tensor.matmul(out=hp2[0:64, :], lhsT=w2_t[0:64, :],
                                 rhs=h1[0:64, :], start=True, stop=True)
                nc.tensor.matmul(out=hp2[64:128, :], lhsT=w2_t[64:128, :],
                                 rhs=h1[64:128, :], start=True, stop=True)
                h2 = mp2.tile([P, 512], f16, tag="h2")
                nc.scalar.activation(out=h2[:], in_=hp2[:], func=SILU)
                hp3 = psH.tile([P, 512], f32, tag="hp")
                nc.tensor.matmul(out=hp3[0:64, :], lhsT=w3_t[0:64, :],
                                 rhs=h2[0:64, :], start=True, stop=True)
                nc.tensor.matmul(out=hp3[64:128, :], lhsT=w3_t[64:128, :],
                                 rhs=h2[64:128, :], start=True, stop=True)
                h3 = mp.tile([P, 512], f16, tag="h3")
                nc.scalar.activation(out=h3[:], in_=hp3[:], func=SILU)
                return h3

            h3s = []
            for g4 in range(N_NODE_TILES // 4):
                xT16 = up_sb.tile([P, 16 * P], f16, tag="xT16")
                nc.sync.dma_start(out=xT16[:], in_=nfT[:, g4, :, :]
                                  .rearrange("p b n -> p (b n)"))
                usb = up_sb.tile([P, 4, 512], f16, tag="usb")
                for q in range(4):
                    nt = g4 * 4 + q
                    ups = psW.tile([P, 512], f32, tag="w512", name="ups")
                    for b in range(4):
                        nc.tensor.matmul(
                            out=ups[:, b * P:(b + 1) * P],
                            lhsT=xT16[:, b * 4 * P + q * P:b * 4 * P + q * P + P],
                            rhs=wup_t[:, b * P:(b + 1) * P], start=True,
                            stop=True)
                    if nt % 2 == 0:
                        nc.vector.tensor_copy(out=usb[:, q, :], in_=ups[:])
                    else:
                        nc.scalar.copy(out=usb[:, q, :], in_=ups[:])
                nc.sync.dma_start(out=table[:, g4 * 4:g4 * 4 + 4, :], in_=usb[:])
                while len(h3s) < (g4 + 1) * nsc // (N_NODE_TILES // 4):
                    h3s.append(emit_mlp(len(h3s)))
            while len(h3s) < nsc:
                h3s.append(emit_mlp(len(h3s)))

            # ---- Phase B: superchunks of 8 edge chunks ----
            acc = None
            for s in range(nsc):
                idx = scp.tile([P, ESC // 16], i16, tag="idx")
                nc.sync.dma_start(out=idx[:], in_=idxd[s])
                g = scp.tile([P, SC, 512], f16, tag="g")
                nc.gpsimd.dma_gather(g[:],
                                     table[:].rearrange("p t c -> (p t) c"),
                                     idx[:], ESC, ESC, 512, queue_num=s % 2)
                ohg = scp.tile([P, SC, 4, P], f16, tag="ohg")
                nc.sync.dma_start(
                    out=ohg[:], in_=ohd[s].rearrange("p (j b c) -> p j b c",
                                                     b=4, c=P))
                h3 = h3s[s]

                for j in range(SC):
                    t, ci, n = sched[s * SC + j]
                    if ci == 0:
                        acc = psA.tile([P, 1024], f32, tag="acc")
                        # bank B holds 4 interleaved accumulation regions;
                        # zero-init once (closed group), then start=False
                        nc.tensor.matmul(out=acc[:, 512:1024], lhsT=zt[:],
                                         rhs=wlin_t[:], start=True, stop=True,
                                         skip_group_check=True)

                    # per-edge TP weights (w4 col order [w0 w2 w3 w1])
                    base = 64 * (j // 4)
                    tpw = psW.tile([P, 512], f32, tag="w512", name="tpw")
                    nc.tensor.matmul(
                        out=tpw[:],
                        lhsT=h3[base:base + 64,
                                128 * (j % 4):128 * (j % 4) + 128],
                        rhs=w4_t[base:base + 64, :], start=True, stop=True)
                    wt = chp.tile([P, 512], f16, tag="wt")
                    nc.scalar.copy(out=wt[:], in_=tpw[:])

                    # rhs products r = [ss*w0 | vs*w3 | ss*w2 | vs*w1]
                    gj = g[:, j, :]
                    r = chp.tile([P, 1024], f16, tag="r")
                    nc.vector.tensor_tensor(
                        out=r[:].rearrange("p (a u) -> p a u", u=P)[:, 0:8:4, :],
                        in0=gj[:, 0:P].rearrange("p (o u) -> p o u", o=1)
                            .to_broadcast([P, 2, P]),
                        in1=wt[:, 0:256].rearrange("p (a u) -> p a u", u=P),
                        op=MUL_)
                    nc.vector.tensor_tensor(
                        out=r[:, P:4 * P].rearrange("p (m u) -> p m u", u=P),
                        in0=gj[:, P:4 * P].rearrange("p (m u) -> p m u", u=P),
                        in1=wt[:, 256:384].rearrange("p (o u) -> p o u", o=1)
                            .to_broadcast([P, 3, P]),
                        op=MUL_)
                    nc.vector.tensor_tensor(
                        out=r[:, 5 * P:8 * P].rearrange("p (m u) -> p m u", u=P),
                        in0=gj[:, P:4 * P].rearrange("p (m u) -> p m u", u=P),
                        in1=wt[:, 384:512].rearrange("p (o u) -> p o u", o=1)
                            .to_broadcast([P, 3, P]),
                        op=MUL_)

                    # scatter: acc layout [m0a | m1b(3) | m0b | m1a(3)]
                    nc.tensor.matmul(out=acc[:, 0:512], lhsT=ohg[:, j, 0, :],
                                     rhs=r[:, 0:512], start=(ci == 0),
                                     stop=(ci == n - 1), skip_group_check=True)
                    for m in range(3):
                        nc.tensor.matmul(
                            out=acc[:, 512:640], lhsT=ohg[:, j, 1 + m, :],
                            rhs=r[:, 5 * P + m * P:6 * P + m * P],
                            start=False,
                            stop=(ci == n - 1 and m == 2),
                            skip_group_check=True)
                        nc.tensor.matmul(
                            out=acc[:, 640 + m * P:768 + m * P],
                            lhsT=ohg[:, j, 1 + m, :], rhs=r[:, 512:640],
                            start=False, stop=(ci == n - 1),
                            skip_group_check=True)

                    if ci == n - 1:
                        # ---- flush node tile t ----
                        msg = fp.tile([P, 1024], f16, tag="msg")
                        nc.vector.tensor_copy(out=msg[:, 0:512], in_=acc[:, 0:512])
                        nc.scalar.copy(out=msg[:, 512:1024], in_=acc[:, 512:1024])
                        psT = psF.tile([P, 1024], f16, tag="psT")
                        for b in range(8):
                            nc.tensor.transpose(
                                out=psT[:, b * P:(b + 1) * P],
                                in_=msg[:, b * P:(b + 1) * P], identity=ident_t[:])
                        msgT = fp.tile([P, 1024], f16, tag="msgT")
                        nc.vector.tensor_copy(out=msgT[:, 0:512], in_=psT[:, 0:512])
                        nc.scalar.copy(out=msgT[:, 512:1024], in_=psT[:, 512:1024])
                        fin = psF.tile([P, 512], f32, tag="fin")
                        nc.tensor.matmul(out=fin[:], lhsT=zt[:], rhs=wlin_t[:],
                                         start=True, stop=True,
                                         skip_group_check=True)
                        nc.tensor.matmul(out=fin[:, 0:P], lhsT=msgT[:, 0:P],
                                         rhs=wlin_t[:, 0:P], start=False,
                                         stop=False, skip_group_check=True)
                        nc.tensor.matmul(out=fin[:, 0:P], lhsT=msgT[:, 4 * P:5 * P],
                                         rhs=wlin_t[:, P:2 * P], start=False,
                                         stop=True, skip_group_check=True)
                        for m in range(3):
                            nc.tensor.matmul(
                                out=fin[:, (1 + m) * P:(2 + m) * P],
                                lhsT=msgT[:, (5 + m) * P:(6 + m) * P],
                                rhs=wlin_t[:, 2 * P:3 * P], start=False,
                                stop=False, skip_group_check=True)
                            nc.tensor.matmul(
                                out=fin[:, (1 + m) * P:(2 + m) * P],
                                lhsT=msgT[:, (1 + m) * P:(2 + m) * P],
                                rhs=wlin_t[:, 3 * P:4 * P], start=False,
                                stop=True, skip_group_check=True)
                        ot = fp.tile([P, 512], f32, tag="ot")
                        nc.vector.tensor_copy(out=ot[:, 0:256], in_=fin[:, 0:256])
                        nc.scalar.copy(out=ot[:, 256:512], in_=fin[:, 256:512])
                        nc.sync.dma_start(out=outd[t * P:(t + 1) * P, :], in_=ot[:])

    nc.compile()
    return nc


def _host_prep(inputs):
    nf = np.asarray(inputs["node_feats"], dtype=np.float32)
    ea = np.asarray(inputs["edge_attrs"], dtype=np.float32)
    ef = np.asarray(inputs["edge_feats"], dtype=np.float32)
    snd = np.asarray(inputs["sender"]).astype(np.int64)
    rcv = np.asarray(inputs["receiver"]).astype(np.int64)

    inv = 1.0 / math.sqrt(MUL)
    inv2 = 1.0 / math.sqrt(2 * MUL)
    c = 1.0 / math.sqrt(MUL)
    c3 = 1.0 / math.sqrt(3.0 * MUL)

    # node feats fp16, block-transposed, per-partition contiguous groups of 4
    s = nf[:, :MUL]
    v = nf[:, MUL:].reshape(-1, MUL, 3)
    nfT = np.zeros((512, NODE_PAD), np.float16)
    nfT[0:128, :N_NODES] = s.T
    for m in range(3):
        nfT[128 * (1 + m):128 * (2 + m), :N_NODES] = v[:, :, m].T
    nfTP = np.ascontiguousarray(
        nfT.reshape(4, P, N_NODE_TILES // 4, 512).transpose(1, 2, 0, 3))

    wup = np.zeros((P, 512), np.float16)
    wup[:, 0:128] = (np.asarray(inputs["W_up0"]) * inv).astype(np.float16)
    w_up1 = (np.asarray(inputs["W_up1"]) * inv).astype(np.float16)
    for m in range(3):
        wup[:, 128 * (1 + m):128 * (2 + m)] = w_up1

    def dup64(w):
        out = np.zeros((P, w.shape[1]), np.float16)
        out[0:w.shape[0]] = w
        out[64:64 + w.shape[0]] = w
        return out

    w1 = dup64((np.asarray(inputs["mlp_w1"]) / math.sqrt(N_RADIAL)
                ).astype(np.float16))
    w2 = dup64((np.asarray(inputs["mlp_w2"]) / math.sqrt(HIDDEN)
                ).astype(np.float16))
    w3 = dup64((np.asarray(inputs["mlp_w3"]) / math.sqrt(HIDDEN)
                ).astype(np.float16))
    w4 = np.asarray(inputs["mlp_w4"]) / math.sqrt(HIDDEN)
    w4 = w4 * np.concatenate([np.full(128, c), np.full(128, c3),
                              np.full(128, c), np.full(128, c)])
    # reorder col blocks [w0 w1 w2 w3] -> [w0 w2 w3 w1]
    w4 = dup64(np.concatenate([w4[:, 0:128], w4[:, 256:384], w4[:, 384:512],
                               w4[:, 128:256]], axis=1).astype(np.float16))
    wlin = np.zeros((P, 512), np.float16)
    lin0 = (np.asarray(inputs["W_lin0"]) * inv2 / 10.0).astype(np.float16)
    lin1 = (np.asarray(inputs["W_lin1"]) * inv2 / 10.0).astype(np.float16)
    wlin[:, 0:128] = lin0[:128]
    wlin[:, 128:256] = lin0[128:]
    wlin[:, 256:384] = lin1[:128]
    wlin[:, 384:512] = lin1[128:]

    ident = np.eye(P, dtype=np.float16)

    core_of = rcv // NODES_PER_CORE
    tile_of = (rcv % NODES_PER_CORE) // P
    sizes = np.zeros((NCORES, TILES_PER_CORE), np.int64)
    np.add.at(sizes, (core_of, tile_of), 1)
    c_prof = [max(1, int(math.ceil(sizes[:, t].max() / P)))
              for t in range(TILES_PER_CORE)]
    rem = sum(c_prof) % SC
    if rem:
        c_prof[-1] += SC - rem
    c_prof = tuple(c_prof)
    nch = sum(c_prof)
    ne_pad = nch * P
    nsc = nch // SC

    order = np.lexsort((rcv, tile_of, core_of))
    er_all = np.zeros((NCORES, ne_pad, 4), np.float16)
    idx_all = np.zeros((NCORES, ne_pad), np.int16)
    ridx_all = np.zeros((NCORES, ne_pad), np.int64)
    eft_all = np.zeros((NCORES, ne_pad, N_RADIAL), np.float16)

    starts = np.concatenate([[0], np.cumsum(np.asarray(c_prof)) * P])[:-1]
    flat_sizes = sizes.reshape(-1)
    run_start = np.concatenate([[0], np.cumsum(flat_sizes)])[:-1].reshape(
        NCORES, TILES_PER_CORE)

    for cidx in range(NCORES):
        for t in range(TILES_PER_CORE):
            n = int(sizes[cidx, t])
            if n == 0:
                continue
            e = order[run_start[cidx, t]:run_start[cidx, t] + n]
            s0 = int(starts[t])
            er_all[cidx, s0:s0 + n, :] = ea[e].astype(np.float16)
            ridx_all[cidx, s0:s0 + n] = (rcv[e] % NODES_PER_CORE) - t * P
            # remap node id for the partition-major table layout
            idx_all[cidx, s0:s0 + n] = ((snd[e] % P) * N_NODE_TILES
                                        + snd[e] // P).astype(np.int16)
            eft_all[cidx, s0:s0 + n, :] = ef[e].astype(np.float16)

    # eftd [nsc, 16, 512]: rows 0:8 edges 0:512, rows 8:16 edges 512:1024
    eftd_all = eft_all.reshape(NCORES, nsc, 2, ESC // 2, N_RADIAL).transpose(
        0, 1, 2, 4, 3).reshape(NCORES, nsc, 2 * N_RADIAL, ESC // 2)
    eftd_all = np.ascontiguousarray(eftd_all)
    # idxd [nsc, 128, 64] int16: wrapped in 16 partitions, replicated x8
    idx16 = idx_all.reshape(NCORES, nsc, ESC // 16, 16).transpose(0, 1, 3, 2)
    idx16 = np.ascontiguousarray(np.tile(idx16, (1, 1, 8, 1)))
    # ohd [nsc, 128, 8*4*128] f16: attr-prescaled one-hot rows
    oh_all = np.zeros((NCORES, ne_pad, 4, P), np.float16)
    np.put_along_axis(oh_all.reshape(-1, 4, P),
                      ridx_all.reshape(-1, 1, 1).repeat(4, axis=1),
                      er_all.reshape(-1, 4, 1), axis=2)
    ohd_all = oh_all.reshape(NCORES, nsc, SC, P, 4, P).transpose(0, 1, 3, 2, 4, 5)
    ohd_all = np.ascontiguousarray(ohd_all).reshape(NCORES, nsc, P, SC * 4 * P)

    common = dict(nfT=nfTP, wup=wup, w1d=w1, w2d=w2, w3d=w3, w4d=w4,
                  wlind=wlin, identd=ident)
    in_maps = []
    for cidx in range(NCORES):
        m = dict(common)
        m.update(eftd=eftd_all[cidx], idxd=idx16[cidx], ohd=ohd_all[cidx])
        in_maps.append(m)
    return c_prof, in_maps


def _unshard(results):
    out = np.empty((N_NODES, 512), np.float32)
    for cidx in range(NCORES):
        lo = cidx * NODES_PER_CORE
        hi = min((cidx + 1) * NODES_PER_CORE, N_NODES)
        if lo >= N_NODES:
            break
        o = results[cidx]["outd"][:hi - lo]
        out[lo:hi, :128] = o[:, :128]
        out[lo:hi, 128:] = o[:, 128:].reshape(-1, 3, 128).transpose(
            0, 2, 1).reshape(-1, 384)
    return out


def kernel(**inputs):
    from concourse.bass_utils import run_bass_kernel_spmd

    c_prof, in_maps = _host_prep(inputs)
    if c_prof not in _CACHE:
        _CACHE[c_prof] = _build(c_prof)
    nc = _CACHE[c_prof]

    trace = bool(os.environ.get("KERNEL_TRACE"))
    if trace:
        import sys, types
        import concourse.bass_utils as bu
        try:
            import antenv.axon_hooks  # noqa
        except ImportError:
            import trn_agent_boot.trn_boot as tb
            hooks = types.ModuleType("antenv.axon_hooks")
            hk = tb._ntff_profile_via_ctypes("/opt/axon/libaxon_pjrt.so")
            hooks.get_axon_ntff_profile_hook = lambda: hk
            hooks.set_axon_ntff_profile_hook = lambda h: None
            sys.modules["antenv.axon_hooks"] = hooks
        bu.upload_artifacts = lambda d: d

    res = run_bass_kernel_spmd(nc, in_maps, list(range(NCORES)), trace=trace)
    if trace and res.exec_time_ns is not None:
        print(f"HW exec time: {res.exec_time_ns} ns")
        if res.instructions_and_trace:
            print(f"trace: {res.instructions_and_trace[1]}")

    return _unshard(res.results)

